# revision 31
# baseline (speedup 1.0000x reference)
"""GRNN (Nadaraya-Watson + linear head) Trainium2 Bass kernel, 8-way row-parallel.

Math: for x [N,D], the reference computes
    sqd_ij = ||x_i||^2 + ||x_j||^2 - 2 x_i.x_j
    w_ij   = exp(-sqd_ij / (2 sigma^2)),  w~ = w / rowsum(w)
    out    = (w~ @ x) @ W.T + b
The exp(-||x_i||^2/2s^2) factor is constant per row i and cancels in the
normalization, so w~ is a softmax over z_ij = (2 x_i.x_j - ||x_j||^2)/(2 s^2).
z is O(0.5) here, so no max-subtraction is needed; EPS=1e-8 is ~1e-12 of the
row sum and is dropped.

Sharding: rows of x are split across 8 cores (1024 rows each); every core
streams the full x (replicated in its HBM) flash-attention style.

Per core, everything is laid out so no on-device transpose is ever needed:
  - G^T[j, i] blocks     = xfT_chunk.T @ xbT_chunk          (PSUM, K=D in 4 chunks)
  - w^T[j, i] = Exp(G^T * 1/1024 + bias_j), bias_j = -sq_j/2048 per-partition
  - y^T[d, i] += xf[j-chunk, d-chunk].T @ w^T               (accum over all j)
  - r[1, i]   += ones.T @ w^T                               (softmax denominator)
  - out[i, o] = (y^T_chunk.T @ W^T_chunk) * (1/r_i)
"""

import numpy as np
import ml_dtypes

BF16 = ml_dtypes.bfloat16

# Problem geometry (hardcoded per spec: x [8192, 512], W [512, 512], b [512])
N = 8192          # total rows of x == number of kernel-weight columns
D = 512           # feature dim
O = 512           # output dim
NCORES = 8
MB = N // NCORES  # rows per core (1024)
JC = 128          # j-chunk (partition dim of w^T tiles)
NJ = N // JC      # 64 j-chunks
DC = 128          # d-chunk
NDC = D // DC     # 4 d-chunks
IHW = 512         # i-half width (one PSUM bank of fp32)
NIH = MB // IHW   # 2 i-halves per core
NT = IHW // 128   # 4 i-tiles per half

SIGMA = 32.0
INV_2S2 = 1.0 / (2.0 * SIGMA * SIGMA)          # 1/2048
EXP_SCALE = 2.0 * INV_2S2                      # 1/1024 (z = 2G/2048 - sq/2048)

_CACHE = {}
DEBUG = False


def _build_nc():
    import concourse.bacc as bacc
    import concourse.mybir as mybir
    import concourse.tile as tile

    fp32 = mybir.dt.float32
    bf16 = mybir.dt.bfloat16

    nc = bacc.Bacc("TRN2", target_bir_lowering=False, debug=False, num_devices=NCORES)

    fp8 = mybir.dt.float8e4
    xfT = nc.dram_tensor("xfT", [NDC, DC, N], fp8, kind="ExternalInput")
    xf = nc.dram_tensor("xf", [N, D], bf16, kind="ExternalInput")
    xbT = nc.dram_tensor("xbT", [NDC, DC, MB], fp8, kind="ExternalInput")
    wTh = nc.dram_tensor("wTh", [NDC, DC, O], bf16, kind="ExternalInput")
    out = nc.dram_tensor("out", [MB, O], fp32, kind="ExternalOutput")
    if DEBUG:
        dbg_sqb = nc.dram_tensor("dbg_sqb", [JC, NJ], fp32, kind="ExternalOutput")
        dbg_w = nc.dram_tensor("dbg_w", [JC, IHW], fp32, kind="ExternalOutput")
        dbg_r = nc.dram_tensor("dbg_r", [IHW], fp32, kind="ExternalOutput")
        dbg_y = nc.dram_tensor("dbg_y", [DC, NDC, IHW], fp32, kind="ExternalOutput")

    # j-chunks per DMA load group: small first groups so PE starts early
    GROUPS = [1, 1, 1, 1, 2, 2, 4, 4] + [8] * ((NJ - 16) // 8)
    GROUPS = [g for g in GROUPS if g][:sum(1 for _ in GROUPS)]
    if NJ == 8:
        GROUPS = [1, 1, 1, 1, 2, 2]
    assert sum(GROUPS) == NJ

    with tile.TileContext(nc) as tc:
        with (
            tc.tile_pool(name="big", bufs=1) as big,
            tc.tile_pool(name="wpool", bufs=6) as wpool,
            tc.tile_pool(name="ypool", bufs=2) as ypool,
            tc.tile_pool(name="misc", bufs=2) as misc,
            tc.tile_pool(name="gps", bufs=2, space="PSUM") as gps,
            tc.tile_pool(name="yps", bufs=1, space="PSUM") as yps,
            tc.tile_pool(name="rps", bufs=1, space="PSUM") as rps,
            tc.tile_pool(name="hps", bufs=1, space="PSUM") as hps,
        ):
            # ---- resident SBUF tensors ----
            # single 3D tiles + consolidated DMAs: one DIRECT2D dispatch per
            # group (the Sync sequencer costs ~0.6us per dma_start dispatch)
            ones_sb = big.tile([JC, 4], bf16, name="ones_sb", tag="ones")
            nc.vector.memset(ones_sb[:], 1.0)
            idone_sb = big.tile([1, 1], fp32, name="idone_sb", tag="idone")
            nc.vector.memset(idone_sb[:], 1.0)

            xbT_sb = big.tile([DC, NDC, MB], fp8, name="xbT_sb", tag="xbT")
            # split across queues: the first G matmul gates on this load
            for c in range(NDC):
                nc.sync.dma_start(xbT_sb[:, c, 0:IHW], xbT[c, :, 0:IHW])

            xfT_sb = big.tile([DC, NDC, N], fp8, name="xfT_sb", tag="xfT")
            xf_sb = [big.tile([JC, D], bf16, name=f"xf_sb{j}", tag=f"xf{j}")
                     for j in range(NJ)]
            # interleave loads group-wise so early j-chunks land first
            jg = 0
            for g in GROUPS:
                j0, j1 = jg * JC, (jg + g) * JC
                nc.sync.dma_start(
                    xfT_sb[:, :, j0:j1],
                    xfT[:, :, j0:j1].rearrange("c p j -> p c j"))
                for j in range(jg, jg + g):
                    nc.sync.dma_start(xf_sb[j][:], xf[j * JC:(j + 1) * JC, :])
                if jg == 0 and IHW < MB:
                    nc.sync.dma_start(
                        xbT_sb[:, :, IHW:MB],
                        xbT[:, :, IHW:MB].rearrange("c p i -> p c i"))
                jg += g

            wTh_sb = big.tile([DC, NDC, O], bf16, name="wTh_sb", tag="wTh")
            nc.sync.dma_start(wTh_sb[:], wTh[:].rearrange("c p o -> p c o"))

            sqb_sb = big.tile([JC, NJ], fp32, name="sqb_sb", tag="sqb")

            yps_t = None
            for ih in range(NIH):
                i0 = ih * IHW

                # ---- streaming j-loop: G^T -> exp -> y^T/r accumulation ----
                # one tile per d-chunk: each accumulator must own a full PSUM
                # bank, since matmul start=True zeroes whole 2KB zero-regions
                yps_t = [yps.tile([DC, IHW], fp32, name=f"y_ps{ih}_{c}",
                                  tag=f"y{c}") for c in range(NDC)]
                r_ps = rps.tile([4, IHW], fp32, name=f"r_ps{ih}", tag="r")

                def g_block(jc, ih=ih, i0=i0):
                    # fp8 DoubleRow: each matmul contracts 2 d-chunks (K=256)
                    g = gps.tile([JC, IHW], fp32, name=f"g_ps{ih}_{jc}", tag="g")
                    for c2 in range(NDC // 2):
                        nc.tensor.matmul(
                            g[:],
                            xfT_sb[:, 2 * c2:2 * c2 + 2, jc * JC:(jc + 1) * JC],
                            xbT_sb[:, 2 * c2:2 * c2 + 2, i0:i0 + IHW],
                            start=(c2 == 0), stop=(c2 == NDC // 2 - 1),
                            perf_mode=mybir.MatmulPerfMode.DoubleRow,
                        )
                    if ih == 0:
                        # bias_j = -||x_j||^2 / 2048, fused square+reduce on DVE
                        sqd = misc.tile([JC, D], bf16, name=f"sqd{jc}", tag="sqd")
                        nc.vector.scalar_tensor_tensor(
                            sqd[:], xf_sb[jc][:], -INV_2S2, xf_sb[jc][:],
                            op0=mybir.AluOpType.mult, op1=mybir.AluOpType.mult,
                            accum_out=sqb_sb[:, jc:jc + 1],
                        )
                    w = wpool.tile([JC, IHW], bf16, name=f"w_sb{ih}_{jc}", tag="w")
                    nc.scalar.activation(
                        w[:], g[:], mybir.ActivationFunctionType.Exp,
                        bias=sqb_sb[:, jc:jc + 1], scale=EXP_SCALE,
                    )
                    if DEBUG and ih == 0 and jc == 0:
                        wf = misc.tile([JC, IHW], fp32, name="wf_dbg", tag="wf_dbg")
                        nc.vector.tensor_copy(wf[:], w[:])
                        nc.sync.dma_start(dbg_w[:], wf[:])
                    return w

                RACC = 8      # j-chunks of w pre-summed (on DVE) per r-matmul
                NQ = NJ // RACC

                def r_mm(q, wsum):
                    # softmax denominator: ones.T @ sum(w); the tree pre-sum
                    # runs on the otherwise-idle DVE, so PE pays one r-matmul
                    # per RACC j-chunks
                    nc.tensor.matmul(
                        r_ps[:], ones_sb[:], wsum[:],
                        start=(q == 0), stop=(q == NQ - 1),
                        skip_group_check=True,
                    )

                w_tiles = {0: g_block(0)}
                acc_tiles = {}   # (level, idx) -> partial sum tile
                pending_r = []

                def acc_put(level, idx, t, ih=ih):
                    # binary tree: level L holds sums of 2^L w tiles
                    if 2 ** level == RACC:
                        pending_r.append((idx, t))
                        return
                    if (level, idx ^ 1) in acc_tiles:
                        sib = acc_tiles.pop((level, idx ^ 1))
                        s = misc.tile([JC, IHW], bf16,
                                      name=f"acc{ih}_{level}_{idx}",
                                      tag=f"acc{level}", bufs=3)
                        nc.vector.tensor_add(s[:], sib[:], t[:])
                        acc_put(level + 1, idx // 2, s)
                    else:
                        acc_tiles[(level, idx)] = t

                for jc in range(NJ):
                    # emit next G block first so PE never waits on ACT's exp
                    if jc + 1 < NJ:
                        w_tiles[jc + 1] = g_block(jc + 1)
                    if pending_r:
                        r_mm(*pending_r.pop(0))
                    w = w_tiles.pop(jc)
                    for c in range(NDC):
                        nc.tensor.matmul(
                            yps_t[c][:],
                            xf_sb[jc][:, c * DC:(c + 1) * DC],
                            w[:],
                            start=(jc == 0), stop=(jc == NJ - 1),
                            skip_group_check=True,
                        )
                    acc_put(0, jc, w)
                while pending_r:
                    r_mm(*pending_r.pop(0))

                # ---- epilogue: stage y^T, transpose r, head matmuls ----
                r_row = misc.tile([1, IHW], fp32, name=f"r_row{ih}", tag="r_row")
                nc.scalar.copy(r_row[:], r_ps[0:1, :])

                ysb = [ypool.tile([DC, IHW], bf16, name=f"ysb{ih}_{c}", tag=f"y{c}")
                       for c in range(NDC)]
                for c in range(NDC):
                    # split the staging copies across ACT and DVE
                    if c < 2:
                        nc.scalar.copy(ysb[c][:], yps_t[c][:])
                    else:
                        nc.vector.tensor_copy(ysb[c][:], yps_t[c][:])
                if DEBUG and ih == 0:
                    nc.sync.dma_start(dbg_sqb[:], sqb_sb[:])
                    nc.sync.dma_start(dbg_r[:], r_row[:])
                    yf = misc.tile([DC, NDC * IHW], fp32, name="yf_dbg", tag="yf_dbg")
                    for c in range(NDC):
                        nc.vector.tensor_copy(
                            yf[:, c * IHW:(c + 1) * IHW], yps_t[c][:])
                    nc.sync.dma_start(dbg_y[:], yf[:])
                # transpose r [1, IHW] -> [128, NT] via PE transpose-mode
                rt = gps.tile([128, IHW], fp32, name=f"rt{ih}", tag="g")
                for t in range(NT):
                    nc.tensor.matmul(
                        rt[:, t:t + 1],
                        r_row[0:1, t * 128:(t + 1) * 128],
                        idone_sb[:],
                        is_transpose=True,
                        start=(t == 0), stop=(t == NT - 1),
                        skip_group_check=True,
                    )
                recip = misc.tile([128, NT], fp32, name=f"recip{ih}", tag="recip")
                nc.vector.reciprocal(recip[:], rt[:, 0:NT])

                for t in range(NT):
                    # on the last half the g banks are free: double-buffer the
                    # head psum across hps/gps to overlap the i-tiles
                    if ih == NIH - 1 and t % 2 == 1:
                        hp = gps.tile([128, O], fp32, name=f"h_ps{ih}_{t}",
                                      tag="g")
                    else:
                        hp = hps.tile([128, O], fp32, name=f"h_ps{ih}_{t}",
                                      tag="h")
                    for c in range(NDC):
                        nc.tensor.matmul(
                            hp[:],
                            ysb[c][:, t * 128:(t + 1) * 128],
                            wTh_sb[:, c, :],
                            start=(c == 0), stop=(c == NDC - 1),
                        )
                    osb = misc.tile([128, O], fp32, name=f"osb{ih}_{t}", tag="osb")
                    nc.vector.tensor_scalar_mul(osb[:], hp[:], recip[:, t:t + 1])
                    nc.sync.dma_start(
                        out[i0 + t * 128:i0 + (t + 1) * 128, :], osb[:])

    nc.compile()
    return nc


def _get_nc():
    if "nc" not in _CACHE:
        _CACHE["nc"] = _build_nc()
    return _CACHE["nc"]


def kernel(x: np.ndarray, W: np.ndarray, b: np.ndarray) -> np.ndarray:
    from concourse import bass_utils

    x = np.asarray(x, dtype=np.float32)
    W = np.asarray(W, dtype=np.float32)
    b = np.asarray(b, dtype=np.float32)

    import concourse.mybir as mybir
    FP8 = mybir.dt.np(mybir.dt.float8e4)

    xT = np.ascontiguousarray(x.T)
    xfT_np = xT.reshape(NDC, DC, N).astype(FP8)
    xf_np = x.astype(BF16)
    wTh_np = np.ascontiguousarray(W.T).reshape(NDC, DC, O).astype(BF16)

    in_maps = []
    for k in range(NCORES):
        xbT_np = np.ascontiguousarray(
            xT[:, k * MB:(k + 1) * MB]).reshape(NDC, DC, MB).astype(FP8)
        in_maps.append({"xfT": xfT_np, "xf": xf_np, "xbT": xbT_np, "wTh": wTh_np})

    nc = _get_nc()
    br = bass_utils.run_bass_kernel_spmd(nc, in_maps, core_ids=list(range(NCORES)))
    _CACHE["last_results"] = br

    out = np.concatenate([br.results[k]["out"] for k in range(NCORES)], axis=0)
    return (out + b[None, :]).astype(np.float32)


# revision 32
# speedup vs baseline: 1.0120x; 1.0120x over previous
"""GRNN (Nadaraya-Watson + linear head) Trainium2 Bass kernel, 8-way row-parallel.

Math: for x [N,D], the reference computes
    sqd_ij = ||x_i||^2 + ||x_j||^2 - 2 x_i.x_j
    w_ij   = exp(-sqd_ij / (2 sigma^2)),  w~ = w / rowsum(w)
    out    = (w~ @ x) @ W.T + b
The exp(-||x_i||^2/2s^2) factor is constant per row i and cancels in the
normalization, so w~ is a softmax over z_ij = (2 x_i.x_j - ||x_j||^2)/(2 s^2).
z is O(0.5) here, so no max-subtraction is needed; EPS=1e-8 is ~1e-12 of the
row sum and is dropped.

Sharding: rows of x are split across 8 cores (1024 rows each); every core
streams the full x (replicated in its HBM) flash-attention style.

Per core, everything is laid out so no on-device transpose is ever needed:
  - G^T[j, i] blocks     = xfT_chunk.T @ xbT_chunk          (PSUM, K=D in 4 chunks)
  - w^T[j, i] = Exp(G^T * 1/1024 + bias_j), bias_j = -sq_j/2048 per-partition
  - y^T[d, i] += xf[j-chunk, d-chunk].T @ w^T               (accum over all j)
  - r[1, i]   += ones.T @ w^T                               (softmax denominator)
  - out[i, o] = (y^T_chunk.T @ W^T_chunk) * (1/r_i)
"""

import numpy as np
import ml_dtypes

BF16 = ml_dtypes.bfloat16

# Problem geometry (hardcoded per spec: x [8192, 512], W [512, 512], b [512])
N = 8192          # total rows of x == number of kernel-weight columns
D = 512           # feature dim
O = 512           # output dim
NCORES = 8
MB = N // NCORES  # rows per core (1024)
JC = 128          # j-chunk (partition dim of w^T tiles)
NJ = N // JC      # 64 j-chunks
DC = 128          # d-chunk
NDC = D // DC     # 4 d-chunks
IHW = 512         # i-half width (one PSUM bank of fp32)
NIH = MB // IHW   # 2 i-halves per core
NT = IHW // 128   # 4 i-tiles per half

SIGMA = 32.0
INV_2S2 = 1.0 / (2.0 * SIGMA * SIGMA)          # 1/2048
EXP_SCALE = 2.0 * INV_2S2                      # 1/1024 (z = 2G/2048 - sq/2048)

_CACHE = {}
DEBUG = False


def _build_nc():
    import concourse.bacc as bacc
    import concourse.mybir as mybir
    import concourse.tile as tile

    fp32 = mybir.dt.float32
    bf16 = mybir.dt.bfloat16

    nc = bacc.Bacc("TRN2", target_bir_lowering=False, debug=False, num_devices=NCORES)

    fp8 = mybir.dt.float8e4
    xfT = nc.dram_tensor("xfT", [NDC, DC, N], fp8, kind="ExternalInput")
    xf = nc.dram_tensor("xf", [N, D], bf16, kind="ExternalInput")
    xbT = nc.dram_tensor("xbT", [NDC, DC, MB], fp8, kind="ExternalInput")
    wTh = nc.dram_tensor("wTh", [NDC, DC, O], bf16, kind="ExternalInput")
    out = nc.dram_tensor("out", [MB, O], fp32, kind="ExternalOutput")
    if DEBUG:
        dbg_sqb = nc.dram_tensor("dbg_sqb", [JC, NJ], fp32, kind="ExternalOutput")
        dbg_w = nc.dram_tensor("dbg_w", [JC, IHW], fp32, kind="ExternalOutput")
        dbg_r = nc.dram_tensor("dbg_r", [IHW], fp32, kind="ExternalOutput")
        dbg_y = nc.dram_tensor("dbg_y", [DC, NDC, IHW], fp32, kind="ExternalOutput")

    # j-chunks per DMA load group: small first groups so PE starts early
    GROUPS = [1, 1, 1, 1, 2, 2, 4, 4] + [8] * ((NJ - 16) // 8)
    GROUPS = [g for g in GROUPS if g][:sum(1 for _ in GROUPS)]
    if NJ == 8:
        GROUPS = [1, 1, 1, 1, 2, 2]
    assert sum(GROUPS) == NJ

    with tile.TileContext(nc) as tc:
        with (
            tc.tile_pool(name="big", bufs=1) as big,
            tc.tile_pool(name="wpool", bufs=6) as wpool,
            tc.tile_pool(name="ypool", bufs=2) as ypool,
            tc.tile_pool(name="misc", bufs=2) as misc,
            tc.tile_pool(name="gps", bufs=2, space="PSUM") as gps,
            tc.tile_pool(name="yps", bufs=1, space="PSUM") as yps,
            tc.tile_pool(name="rps", bufs=1, space="PSUM") as rps,
            tc.tile_pool(name="hps", bufs=1, space="PSUM") as hps,
        ):
            # ---- resident SBUF tensors ----
            # single 3D tiles + consolidated DMAs: one DIRECT2D dispatch per
            # group (the Sync sequencer costs ~0.6us per dma_start dispatch)
            ones_sb = big.tile([JC, 4], bf16, name="ones_sb", tag="ones")
            nc.vector.memset(ones_sb[:], 1.0)
            idone_sb = big.tile([1, 1], fp32, name="idone_sb", tag="idone")
            nc.vector.memset(idone_sb[:], 1.0)

            xbT_sb = big.tile([DC, NDC, MB], fp8, name="xbT_sb", tag="xbT")
            nc.sync.dma_start(
                xbT_sb[:, :, 0:IHW],
                xbT[:, :, 0:IHW].rearrange("c p i -> p c i"))

            xfT_sb = big.tile([DC, NDC, N], fp8, name="xfT_sb", tag="xfT")
            xf_sb = [big.tile([JC, D], bf16, name=f"xf_sb{j}", tag=f"xf{j}")
                     for j in range(NJ)]
            # interleave loads group-wise so early j-chunks land first
            jg = 0
            for g in GROUPS:
                j0, j1 = jg * JC, (jg + g) * JC
                nc.sync.dma_start(
                    xfT_sb[:, :, j0:j1],
                    xfT[:, :, j0:j1].rearrange("c p j -> p c j"))
                for j in range(jg, jg + g):
                    nc.sync.dma_start(xf_sb[j][:], xf[j * JC:(j + 1) * JC, :])
                if jg == 0 and IHW < MB:
                    nc.sync.dma_start(
                        xbT_sb[:, :, IHW:MB],
                        xbT[:, :, IHW:MB].rearrange("c p i -> p c i"))
                jg += g

            wTh_sb = big.tile([DC, NDC, O], bf16, name="wTh_sb", tag="wTh")
            nc.sync.dma_start(wTh_sb[:], wTh[:].rearrange("c p o -> p c o"))

            sqb_sb = big.tile([JC, NJ], fp32, name="sqb_sb", tag="sqb")

            yps_t = None
            for ih in range(NIH):
                i0 = ih * IHW

                # ---- streaming j-loop: G^T -> exp -> y^T/r accumulation ----
                # one tile per d-chunk: each accumulator must own a full PSUM
                # bank, since matmul start=True zeroes whole 2KB zero-regions
                yps_t = [yps.tile([DC, IHW], fp32, name=f"y_ps{ih}_{c}",
                                  tag=f"y{c}") for c in range(NDC)]
                r_ps = rps.tile([4, IHW], fp32, name=f"r_ps{ih}", tag="r")

                def g_block(jc, ih=ih, i0=i0):
                    # fp8 DoubleRow: each matmul contracts 2 d-chunks (K=256)
                    g = gps.tile([JC, IHW], fp32, name=f"g_ps{ih}_{jc}", tag="g")
                    for c2 in range(NDC // 2):
                        nc.tensor.matmul(
                            g[:],
                            xfT_sb[:, 2 * c2:2 * c2 + 2, jc * JC:(jc + 1) * JC],
                            xbT_sb[:, 2 * c2:2 * c2 + 2, i0:i0 + IHW],
                            start=(c2 == 0), stop=(c2 == NDC // 2 - 1),
                            perf_mode=mybir.MatmulPerfMode.DoubleRow,
                        )
                    if ih == 0:
                        # bias_j = -||x_j||^2 / 2048, fused square+reduce on DVE
                        sqd = misc.tile([JC, D], bf16, name=f"sqd{jc}", tag="sqd")
                        nc.vector.scalar_tensor_tensor(
                            sqd[:], xf_sb[jc][:], -INV_2S2, xf_sb[jc][:],
                            op0=mybir.AluOpType.mult, op1=mybir.AluOpType.mult,
                            accum_out=sqb_sb[:, jc:jc + 1],
                        )
                    w = wpool.tile([JC, IHW], bf16, name=f"w_sb{ih}_{jc}", tag="w")
                    nc.scalar.activation(
                        w[:], g[:], mybir.ActivationFunctionType.Exp,
                        bias=sqb_sb[:, jc:jc + 1], scale=EXP_SCALE,
                    )
                    if DEBUG and ih == 0 and jc == 0:
                        wf = misc.tile([JC, IHW], fp32, name="wf_dbg", tag="wf_dbg")
                        nc.vector.tensor_copy(wf[:], w[:])
                        nc.sync.dma_start(dbg_w[:], wf[:])
                    return w

                RACC = 8      # j-chunks of w pre-summed (on DVE) per r-matmul
                NQ = NJ // RACC

                def r_mm(q, wsum):
                    # softmax denominator: ones.T @ sum(w); the tree pre-sum
                    # runs on the otherwise-idle DVE, so PE pays one r-matmul
                    # per RACC j-chunks
                    nc.tensor.matmul(
                        r_ps[:], ones_sb[:], wsum[:],
                        start=(q == 0), stop=(q == NQ - 1),
                        skip_group_check=True,
                    )

                w_tiles = {0: g_block(0)}
                acc_tiles = {}   # (level, idx) -> partial sum tile
                pending_r = []

                def acc_put(level, idx, t, ih=ih):
                    # binary tree: level L holds sums of 2^L w tiles
                    if 2 ** level == RACC:
                        pending_r.append((idx, t))
                        return
                    if (level, idx ^ 1) in acc_tiles:
                        sib = acc_tiles.pop((level, idx ^ 1))
                        s = misc.tile([JC, IHW], bf16,
                                      name=f"acc{ih}_{level}_{idx}",
                                      tag=f"acc{level}", bufs=3)
                        nc.vector.tensor_add(s[:], sib[:], t[:])
                        acc_put(level + 1, idx // 2, s)
                    else:
                        acc_tiles[(level, idx)] = t

                for jc in range(NJ):
                    # emit next G block first so PE never waits on ACT's exp
                    if jc + 1 < NJ:
                        w_tiles[jc + 1] = g_block(jc + 1)
                    if pending_r:
                        r_mm(*pending_r.pop(0))
                    w = w_tiles.pop(jc)
                    for c in range(NDC):
                        nc.tensor.matmul(
                            yps_t[c][:],
                            xf_sb[jc][:, c * DC:(c + 1) * DC],
                            w[:],
                            start=(jc == 0), stop=(jc == NJ - 1),
                            skip_group_check=True,
                        )
                    acc_put(0, jc, w)
                while pending_r:
                    r_mm(*pending_r.pop(0))

                # ---- epilogue: stage y^T, transpose r, head matmuls ----
                r_row = misc.tile([1, IHW], fp32, name=f"r_row{ih}", tag="r_row")
                nc.scalar.copy(r_row[:], r_ps[0:1, :])

                ysb = [ypool.tile([DC, IHW], bf16, name=f"ysb{ih}_{c}", tag=f"y{c}")
                       for c in range(NDC)]
                for c in range(NDC):
                    # split the staging copies across ACT and DVE
                    if c < 2:
                        nc.scalar.copy(ysb[c][:], yps_t[c][:])
                    else:
                        nc.vector.tensor_copy(ysb[c][:], yps_t[c][:])
                if DEBUG and ih == 0:
                    nc.sync.dma_start(dbg_sqb[:], sqb_sb[:])
                    nc.sync.dma_start(dbg_r[:], r_row[:])
                    yf = misc.tile([DC, NDC * IHW], fp32, name="yf_dbg", tag="yf_dbg")
                    for c in range(NDC):
                        nc.vector.tensor_copy(
                            yf[:, c * IHW:(c + 1) * IHW], yps_t[c][:])
                    nc.sync.dma_start(dbg_y[:], yf[:])
                # transpose r [1, IHW] -> [128, NT] via PE transpose-mode
                rt = gps.tile([128, IHW], fp32, name=f"rt{ih}", tag="g")
                for t in range(NT):
                    nc.tensor.matmul(
                        rt[:, t:t + 1],
                        r_row[0:1, t * 128:(t + 1) * 128],
                        idone_sb[:],
                        is_transpose=True,
                        start=(t == 0), stop=(t == NT - 1),
                        skip_group_check=True,
                    )
                recip = misc.tile([128, NT], fp32, name=f"recip{ih}", tag="recip")
                nc.vector.reciprocal(recip[:], rt[:, 0:NT])

                for t in range(NT):
                    # on the last half the g banks are free: double-buffer the
                    # head psum across hps/gps to overlap the i-tiles
                    if ih == NIH - 1 and t % 2 == 1:
                        hp = gps.tile([128, O], fp32, name=f"h_ps{ih}_{t}",
                                      tag="g")
                    else:
                        hp = hps.tile([128, O], fp32, name=f"h_ps{ih}_{t}",
                                      tag="h")
                    for c in range(NDC):
                        nc.tensor.matmul(
                            hp[:],
                            ysb[c][:, t * 128:(t + 1) * 128],
                            wTh_sb[:, c, :],
                            start=(c == 0), stop=(c == NDC - 1),
                        )
                    osb = misc.tile([128, O], fp32, name=f"osb{ih}_{t}", tag="osb")
                    nc.vector.tensor_scalar_mul(osb[:], hp[:], recip[:, t:t + 1])
                    nc.sync.dma_start(
                        out[i0 + t * 128:i0 + (t + 1) * 128, :], osb[:])

    nc.compile()
    return nc


def _get_nc():
    if "nc" not in _CACHE:
        _CACHE["nc"] = _build_nc()
    return _CACHE["nc"]


def kernel(x: np.ndarray, W: np.ndarray, b: np.ndarray) -> np.ndarray:
    from concourse import bass_utils

    x = np.asarray(x, dtype=np.float32)
    W = np.asarray(W, dtype=np.float32)
    b = np.asarray(b, dtype=np.float32)

    import concourse.mybir as mybir
    FP8 = mybir.dt.np(mybir.dt.float8e4)

    xT = np.ascontiguousarray(x.T)
    xfT_np = xT.reshape(NDC, DC, N).astype(FP8)
    xf_np = x.astype(BF16)
    wTh_np = np.ascontiguousarray(W.T).reshape(NDC, DC, O).astype(BF16)

    in_maps = []
    for k in range(NCORES):
        xbT_np = np.ascontiguousarray(
            xT[:, k * MB:(k + 1) * MB]).reshape(NDC, DC, MB).astype(FP8)
        in_maps.append({"xfT": xfT_np, "xf": xf_np, "xbT": xbT_np, "wTh": wTh_np})

    nc = _get_nc()
    br = bass_utils.run_bass_kernel_spmd(nc, in_maps, core_ids=list(range(NCORES)))
    _CACHE["last_results"] = br

    out = np.concatenate([br.results[k]["out"] for k in range(NCORES)], axis=0)
    return (out + b[None, :]).astype(np.float32)
